# revision 45
# baseline (speedup 1.0000x reference)
"""Trainium2 Bass kernel for nn_Conv2DExperimental (MVN-sampled 3x3 conv).

Computation (per the nn.Module):
  L    = tril(weight_L, -1) + softplus(diag(weight_L)) * I      # [O,I,D,D], D=9
  w    = weight_loc + einsum('oiab,oib->oia', L, eps_w)         # [O,I,3,3]
  b    = bias_loc + eps_b * softplus(bias_ro)                   # [O]
  out  = conv2d(x, w, SAME, NCHW) + b
  with O = I = 64 channels, x [32, 64, 224, 224].

Distribution: data-parallel over the batch dim of x (32 images -> 8 cores x 4),
with the weight sampling replicated on every core (it is tiny).

Per-core kernel (row-parity conv, 75% PE utilization):
  - x is host-packed bf16 into SBUF layout [128, 113, 228]: partitions
    (parity q, in-channel), where q=0 slot k holds image row 2k and q=1 slot k
    holds row 2k-1 (staggered), columns padded by 2 on both sides.  Halo
    rows/columns are pre-zeroed on the host, so the kernel needs no memsets.
    Images stream in as 29-slot quarters through a 6-deep tile pool, so the
    HBM reads are consumption-paced instead of bursting against the output
    stores (HBM is ~358 GB/s per core).
  - output psum tiles are [128 = (row-parity p, out-channel), 2 pairs x 224]:
    out row 2k+p.  Per tile, 6 matmuls (2 input row-groups x 3 column shifts)
    apply all 9 taps exactly once per output: lhsT tiles have 3 of 4
    64x64 quadrants live (vs 2 of 4 for the image-paired block-diagonal
    scheme) -> 1.5x less PE time.
  - sampling: the host pre-layouts weight_L as a masked strict-lower
    [O, (b, i, a)] block plus the diagonal [O, (i, a)], and pre-broadcasts
    eps_w to the same shape (pure data reshuffle), so L @ eps is one
    contiguous VectorE multiply + 3 tree adds, and softplus(diag) is a
    contiguous ScalarE op.  The 9 tap matrices are PE-transposed into both
    partition halves at once (weights duplicated side by side), then 8
    batched strided copies assemble the 6 lhsT tiles.
  - ScalarE evacuates PSUM with the bias add fused (bf16 out); output is
    stored in a packed [8-strip, 128, 14, 224] layout the host re-interleaves.
"""

import sys
from contextlib import ExitStack

for _p in ("/opt/trn_rl_repo",):
    if _p not in sys.path:
        sys.path.insert(0, _p)

import numpy as np

import concourse.bass as bass
import concourse.bacc as bacc
import concourse.mybir as mybir
from concourse.tile import TileContext

F32 = mybir.dt.float32
F32R = mybir.dt.float32r
BF16 = mybir.dt.bfloat16
AF = mybir.ActivationFunctionType

N_CORES = 8
O = 64
I = 64
KK = 3
D = KK * KK  # 9
ID = I * D  # 576
HH = 224
WW = 224
NP = HH // 2 + 1  # 113 pair slots
WPAD = WW + 4  # 228: 2 zero cols each side
GSTRIP = 8  # output strips per image
PPG = (HH // 2) // GSTRIP  # 14 output row-pairs per strip
QS = [0, 28, 56, 84]  # x quarter start slots (29 slots each, 1-slot overlap)
QLEN = 29


def build_nc(nb=4, n_w1=26, n_w1b=6, n_w2=11):
    """Build the per-core Bass program. nb: images per core."""
    nc = bacc.Bacc("TRN2", target_bir_lowering=False, debug=False)

    xp_t = nc.dram_tensor("xp", [nb, 128, NP, WPAD], BF16, kind="ExternalInput").ap()
    # 128-partition (rows = (b-half h, o)): 64-partition tensors map to only
    # 8 of the 16 SDMA engines, and the VectorE chain gets all 128 lanes
    wl2_t = nc.dram_tensor("wl2", [128, 4 * ID], BF16, kind="ExternalInput").ap()
    eps3_t = nc.dram_tensor("eps3", [128, 4 * ID], BF16, kind="ExternalInput").ap()
    diag_t = nc.dram_tensor("diag", [O, ID], BF16, kind="ExternalInput").ap()
    epsw_t = nc.dram_tensor("epsw", [O, ID], BF16, kind="ExternalInput").ap()
    wloc_t = nc.dram_tensor("wloc", [O, ID], BF16, kind="ExternalInput").ap()
    ident_t = nc.dram_tensor("ident", [O, O], F32, kind="ExternalInput").ap()
    identb_t = nc.dram_tensor("identb", [128, O], BF16, kind="ExternalInput").ap()
    bias3_t = nc.dram_tensor("bias3", [3, O], F32, kind="ExternalInput").ap()
    out_t = nc.dram_tensor(
        "out", [nb, GSTRIP, 128, PPG, WW], BF16, kind="ExternalOutput"
    ).ap()

    with TileContext(nc) as tc, ExitStack() as stack:
        # ---------------- weight + bias sampling (one-time prologue) --------
        cp = stack.enter_context(tc.tile_pool(name="consts", bufs=1))
        wl2 = cp.tile([128, 8 * ID], BF16, name="wl2", tag="wl2")
        eps3 = cp.tile([128, 8 * ID], BF16, name="eps3", tag="eps3")
        diag = cp.tile([O, ID], BF16, name="diag", tag="diag")
        epsw = cp.tile([O, ID], BF16, name="epsw_s", tag="epsw_s")
        wloc = cp.tile([O, ID], BF16, name="wloc_s", tag="wloc_s")
        ident = cp.tile([O, O], F32, name="ident_s", tag="ident_s")
        identb = cp.tile([128, O], BF16, name="identb_s", tag="identb_s")
        b3 = cp.tile([O, 3], F32, name="b3", tag="b3")
        b3p = cp.tile([3, O], F32, name="b3p", tag="b3p")
        sp = cp.tile([O, ID], BF16, name="sp", tag="sp")
        tmp = cp.tile([O, ID], BF16, name="tmp", tag="tmp")
        prod = cp.tile([128, 8 * ID], BF16, name="prod", tag="prod")
        wsamp = cp.tile([O, ID], BF16, name="wsamp", tag="wsamp")
        # transpose input [(h,o), (dup, i, a)]: h=0 rows carry the b<4
        # partial plus the loc+diag term, h=1 rows the b>=4 partial; same
        # per-partition bytes as the old [64, 2*ID] duplicate tile
        wsampd = cp.tile([128, 2 * ID], BF16, name="wsampd", tag="wsampd")
        bias = cp.tile([128, 1], F32, name="bias", tag="bias")
        sp_b = cp.tile([O, 1], F32, name="sp_b", tag="sp_b")
        # 6 lhsT tiles side by side: [128, 6*128] = A_s (s=0..2), B_s (3..5)
        wts = cp.tile([128, 6 * 128], BF16, name="wts", tag="wts")

        # the three critical sampling tensors lead the sync ring in
        # consumption order (diag gates the softplus, wl2+eps3 the big
        # multiply); everything else trails or rides the scalar ring
        # sampling inputs split across both HWDGE rings so the two big
        # blocks (wl2, eps3) transfer in parallel ahead of everything else
        nc.sync.dma_start(wl2[:, 0 : 4 * ID], wl2_t[:])
        nc.sync.dma_start(epsw[:], epsw_t[:])
        nc.sync.dma_start(ident[:], ident_t[:])
        nc.sync.dma_start(b3p[:], bias3_t[:])
        nc.scalar.dma_start(diag[:], diag_t[:])
        nc.scalar.dma_start(eps3[:, 0 : 4 * ID], eps3_t[:])
        nc.scalar.dma_start(identb[:], identb_t[:])
        nc.scalar.dma_start(wloc[:], wloc_t[:])

        # PE warm-up feed: zero tiles via VectorE (fast, no SWDGE latency).
        # Full 128-partition matmuls: 64-wide ones do NOT trip the HAM clock
        # gate (measured: 90x [64,256] warmup left the PE at 1.2 GHz).
        identr = cp.tile([128, 128], F32R, name="identr", tag="identr")
        junk = cp.tile([128, 448], F32R, name="junk", tag="junk")
        with tc.high_priority():
            nc.vector.memset(identr[:].bitcast(F32), 0.0)
            nc.vector.memset(junk[:].bitcast(F32), 0.0)
        # zero the dead lhsT quadrants (A: q1/p1, B: q0/p0) in one shot
        nc.gpsimd.memset(wts[:].bitcast(F32), 0.0)



        with tc.tile_pool(name="prol", bufs=1, space="PSUM") as wp:
            # HAM needs ~3.4us of sustained full-width matmul activity to
            # lift the PE 1.2 -> 2.4 GHz; these also bridge PE-idle windows
            # while VectorE/ScalarE run the sampling chain.
            warm = wp.tile([128, 448], F32, name="warm")
            for k in range(n_w1):
                nc.tensor.matmul(
                    warm[:], identr[:], junk[:], start=(k == 0), stop=(k == n_w1 - 1)
                )

            # bias3 arrives as [3, 64]; transpose to [64, 3] on the PE (a
            # partition-major DMA of 64x3 elements costs ~17us in descriptors)
            bp_ps = wp.tile([O, 3], F32, name="bp_ps")
            nc.tensor.matmul(bp_ps[:], b3p[:], ident[0:3, 0:3], start=True, stop=True)

            for k in range(n_w1b):
                nc.tensor.matmul(
                    warm[:], identr[:], junk[:], start=(k == 0), stop=(k == n_w1b - 1)
                )

            # ---- VectorE sampling chain (all contiguous bf16) ------------
            # prod[o,(b,i,a)] = wl2 * eps3 elementwise; wl2 is host-masked
            # to the strict-lower taps and eps3 is eps_w host-broadcast over
            # a, so the b-tree-sum IS (tril(L,-1) @ eps).
            nc.vector.tensor_mul(prod[:, 0 : 4 * ID], wl2[:, 0 : 4 * ID],
                                 eps3[:, 0 : 4 * ID])
            nc.vector.tensor_add(prod[:, 0 : 2 * ID], prod[:, 0 : 2 * ID],
                                 prod[:, 2 * ID : 4 * ID])
            nc.vector.tensor_add(prod[:, 0:ID], prod[:, 0:ID], prod[:, ID : 2 * ID])
            nc.vector.tensor_copy(b3[:], bp_ps[:])

            # softplus(diag) on ScalarE: Exp then Ln (ln(e^x + 1)); there is
            # no Softplus LUT in this toolchain.
            nc.scalar.activation(sp[:], diag[:], AF.Exp)
            nc.scalar.activation(sp[:], sp[:], AF.Ln, bias=1.0)

            # wsamp = wloc + softplus(diag)*eps + strict_lower (twice, for
            # the both-halves transpose trick)
            nc.vector.tensor_mul(tmp[:], sp[:], epsw[:])
            nc.vector.tensor_add(wsamp[:], wloc[:], tmp[:])
            # build the transpose input, duplicated free dim in one op each:
            #   X2[h=0] = prod[h=0] + wsamp   X2[h=1] = prod[h=1]
            def dup_ap(t, half, bcast):
                base = t[:]
                return bass.AP(
                    tensor=base.tensor,
                    offset=base.offset + half * O * base.ap[0][0],
                    ap=[[base.ap[0][0], O], [0 if bcast else ID, 2], [1, ID]],
                )

            nc.vector.tensor_tensor(
                dup_ap(wsampd, 0, False), dup_ap(prod, 0, True),
                dup_ap(wsamp, 0, True), mybir.AluOpType.add,
            )
            nc.vector.tensor_copy(dup_ap(wsampd, 1, False), dup_ap(prod, 1, True))

            # ---- tap transposes + lhsT assembly --------------------------
            # T[t][ich,och] = wsamp[och, ich*9+t], written to BOTH partition
            # halves of ptA/ptB at once via the duplicated wsampd free dim.
            # regular matmuls (transpose-mode is a passthrough datapath and
            # would not sum): out = w_a2.T @ [I; I] transposes each tap AND
            # folds the two b-half partials in the contraction
            ptA = wp.tile([128, 5 * O], F32, name="ptA")
            ptB = wp.tile([128, 4 * O], F32, name="ptB")
            for a in range(D):
                w_a2 = bass.AP(
                    tensor=wsampd[:].tensor,
                    offset=wsampd[:].offset + a,
                    ap=[list(p) for p in wsampd[:].ap[:1]] + [[ID, 2], [D, I]],
                )
                dst_pt = ptA if a < 5 else ptB
                c = a if a < 5 else a - 5
                nc.tensor.matmul(
                    dst_pt[:, c * O : (c + 1) * O],
                    w_a2,
                    identb[:],
                    start=(c == 0),
                    stop=(c == (4 if a < 5 else 3)),
                    skip_group_check=True,
                )

            # keep the PE busy while the lhsT copies run
            for k in range(n_w2):
                nc.tensor.matmul(
                    warm[:], identr[:], junk[:], start=(k == 0), stop=(k == n_w2 - 1)
                )

            # batched strided copies (dst stride 128, src stride 64):
            #   A_s: [q0,p0]=T[3+s]  [q0,p1]=T[s]  [q1,p0]=T[s]  [q1,p1]=0
            #   B_s: [q0,p0]=0  [q0,p1]=T[6+s]  [q1,p0]=T[6+s]  [q1,p1]=T[3+s]
            def bcopy(eng, dst_c0, dst_n, src_pt, src_half, src_c0):
                pstr = wts[:].ap[0][0]
                dst = bass.AP(
                    tensor=wts[:].tensor,
                    offset=wts[:].offset + src_half * 64 * pstr + dst_c0,
                    ap=[[pstr, 64], [128, dst_n], [1, O]],
                )
                s_ = src_pt[src_half * 64 : src_half * 64 + 64,
                            src_c0 : src_c0 + dst_n * O]
                src = bass.AP(
                    tensor=s_.tensor, offset=s_.offset,
                    ap=[list(s_.ap[0])] + [[O, dst_n], [1, O]],
                )
                if eng == "v":
                    nc.vector.tensor_copy(dst, src)
                else:
                    nc.scalar.activation(dst, src, AF.Copy)

            bcopy("v", O, 3, ptA, 0, 0)        # A q0,p1 <- T[0..2]
            bcopy("s", 0, 3, ptA, 1, 0)        # A q1,p0 <- T[0..2]
            bcopy("v", 0, 2, ptA, 0, 3 * O)    # A q0,p0 <- T[3..4]
            bcopy("s", 2 * 128, 1, ptB, 0, 0)  # A2 q0,p0 <- T[5]
            bcopy("v", 3 * 128 + O, 3, ptB, 0, O)      # B q0,p1 <- T[6..8]
            bcopy("s", 3 * 128, 3, ptB, 1, O)          # B q1,p0 <- T[6..8]
            bcopy("v", 3 * 128 + O, 2, ptA, 1, 3 * O)  # B0-1 q1,p1 <- T[3..4]
            bcopy("s", 5 * 128 + O, 1, ptB, 1, 0)      # B2 q1,p1 <- T[5]

            # bias = bias_loc + eps_b * softplus(bias_ro)  (off critical path)
            nc.scalar.activation(sp_b[:], b3[:, 1:2], AF.Exp)
            nc.scalar.activation(sp_b[:], sp_b[:], AF.Ln, bias=1.0)
            nc.vector.tensor_mul(sp_b[:], sp_b[:], b3[:, 2:3])
            nc.vector.tensor_add(bias[0:O, :], b3[:, 0:1], sp_b[:])
            nc.scalar.dma_start(bias[O:128, :], bias[0:O, :])

        # ---------------- convolution ---------------------------------------
        # per psum tile t (out rows 4t..4t+3 of one image):
        #   acc[(p,och), (k in {2t,2t+1}, c)] = out row 2k+p
        #   A_s: rhs slots (2t, 2t+1)   B_s: rhs slots (2t+1, 2t+2)
        #   rhs col start = s+1 (packed col cc = image col + 2)
        # bufs=2: only two quarters in flight, so image reads are paced by
        # conv consumption instead of queueing 6.4MB against the stores and
        # the prologue loads (whole-packet round-robin on the SDMA engines)
        xqp = stack.enter_context(tc.tile_pool(name="xq", bufs=2))
        op = stack.enter_context(tc.tile_pool(name="ostrip", bufs=2))
        pp = stack.enter_context(tc.tile_pool(name="acc", bufs=8, space="PSUM"))
        for n in range(nb):
            xq = []
            for q in range(4):
                xt = xqp.tile([128, QLEN, WPAD], BF16, name="xq")
                nc.sync.dma_start(xt[:], xp_t[n, :, QS[q] : QS[q] + QLEN, :])
                xq.append(xt)
            for g in range(GSTRIP):
                os_ = op.tile([128, PPG, WW], BF16, name="os_")
                last = n == nb - 1 and g == GSTRIP - 1
                # psum tiles in pairs, matmuls grouped by lhsT so each
                # stationary load serves two consecutive matmuls
                for prt in ((0, 1), (2, 3), (4, 5), (6,)):
                    accs = []
                    for tt in prt:
                        t = (PPG // 2) * g + tt
                        qi = (t >= 14) + (t >= 28) + (t >= 42)
                        accs.append(
                            (tt, t, xq[qi], 2 * t - QS[qi],
                             pp.tile([128, 2, WW], F32, name="acc"))
                        )
                    for s in range(3):
                        for lhs_c0, off, is_a in (
                            (s * 128, 0, True),
                            ((3 + s) * 128, 1, False),
                        ):
                            for tt, t, xs, lo, acc in accs:
                                nc.tensor.matmul(
                                    acc[:],
                                    wts[:, lhs_c0 : lhs_c0 + 128],
                                    xs[:, lo + off : lo + off + 2,
                                       s + 1 : s + 1 + WW],
                                    start=(s == 0 and is_a),
                                    stop=(s == 2 and not is_a),
                                    skip_group_check=True,
                                )
                    for tt, t, xs, lo, acc in accs:
                        nc.scalar.activation(
                            os_[:, 2 * tt : 2 * tt + 2, :],
                            acc[:],
                            AF.Identity,
                            bias=bias[:, 0:1],
                        )
                        # taper: stream the final strip out in pieces so the
                        # kernel does not end on a full-size store
                        if last and tt in (1, 3, 5):
                            k0, k1 = {1: (0, 4), 3: (4, 8), 5: (8, 12)}[tt]
                            nc.scalar.dma_start(
                                out_t[n, g, :, k0:k1, :], os_[:, k0:k1, :]
                            )
                if last:
                    nc.scalar.dma_start(out_t[n, g, :, 12:PPG, :], os_[:, 12:PPG, :])
                else:
                    nc.scalar.dma_start(out_t[n, g], os_[:])

    nc.compile()
    return nc


_CACHED_NC = None


def _pack_x(x_shard_bf):
    """[nb, 64, 224, 224] bf16 -> [nb, 128, 113, 228] staggered parity pack."""
    nb = x_shard_bf.shape[0]
    xp = np.zeros((nb, 128, NP, WPAD), dtype=x_shard_bf.dtype)
    xp[:, 0:64, 0 : HH // 2, 2 : WW + 2] = x_shard_bf[:, :, 0::2, :]
    xp[:, 64:128, 1 : HH // 2 + 1, 2 : WW + 2] = x_shard_bf[:, :, 1::2, :]
    return xp


def _host_inputs(x_shard, weight_loc, weight_L, bias_loc, bias_ro, eps_w, eps_b):
    import ml_dtypes

    bf = ml_dtypes.bfloat16
    wlf = np.asarray(weight_L, np.float32)  # [O, I, D(a), D(b)]
    mask = np.tril(np.ones((D, D), np.float32), -1)  # [a, b]: a > b
    wl2 = (wlf * mask).transpose(0, 3, 1, 2)[:, 0:8]  # [O, b, I, a]
    dg = np.diagonal(wlf, axis1=2, axis2=3)  # [O, I, D]
    ew = np.asarray(eps_w, np.float32)  # [O, I, D(b)]
    eps3 = np.broadcast_to(
        ew.transpose(0, 2, 1)[:, 0:8, :, None], (O, 8, I, D)
    )  # [O, b, I, a]: eps_w[o,i,b] for every a
    return {
        "xp": _pack_x(np.asarray(x_shard).astype(bf)),
        "wl2": np.ascontiguousarray(
            wl2.reshape(O, 2, 4 * ID).transpose(1, 0, 2).reshape(128, 4 * ID)
        ).astype(bf),
        "eps3": np.ascontiguousarray(
            np.ascontiguousarray(eps3).reshape(O, 2, 4 * ID)
            .transpose(1, 0, 2).reshape(128, 4 * ID)
        ).astype(bf),
        "diag": np.ascontiguousarray(dg.reshape(O, ID)).astype(bf),
        "epsw": np.ascontiguousarray(ew.reshape(O, ID)).astype(bf),
        "wloc": np.ascontiguousarray(
            np.asarray(weight_loc, np.float32).reshape(O, ID)
        ).astype(bf),
        "ident": np.eye(O, dtype=np.float32),
        "identb": np.tile(np.eye(O, dtype=np.float32), (2, 1)).astype(bf),
        "bias3": np.ascontiguousarray(
            np.stack([bias_loc, bias_ro, eps_b]).astype(np.float32)
        ),
    }


def kernel(x, weight_loc, weight_L, bias_loc, bias_ro, eps_w, eps_b):
    global _CACHED_NC
    from concourse.bass_utils import run_bass_kernel_spmd

    x = np.asarray(x, np.float32)
    nb = x.shape[0] // N_CORES
    if _CACHED_NC is None:
        _CACHED_NC = build_nc(nb=nb)
    nc = _CACHED_NC

    import ml_dtypes

    x_bf = x.astype(ml_dtypes.bfloat16)
    in_maps = [
        _host_inputs(
            x_bf[c * nb : (c + 1) * nb],
            np.asarray(weight_loc),
            np.asarray(weight_L),
            np.asarray(bias_loc),
            np.asarray(bias_ro),
            np.asarray(eps_w),
            np.asarray(eps_b),
        )
        for c in range(N_CORES)
    ]
    res = run_bass_kernel_spmd(nc, in_maps, list(range(N_CORES)))
    outs = []
    for c in range(N_CORES):
        o = np.asarray(res.results[c]["out"])  # [nb, 8, 128, 14, 224] bf16
        o = o.reshape(nb, GSTRIP, 2, O, PPG, WW).transpose(0, 3, 1, 4, 2, 5)
        outs.append(o.reshape(nb, O, HH, WW).astype(np.float32))
    return np.concatenate(outs, axis=0)


# revision 46
# speedup vs baseline: 1.1836x; 1.1836x over previous
"""Trainium2 Bass kernel for nn_Conv2DExperimental (MVN-sampled 3x3 conv).

Computation (per the nn.Module):
  L    = tril(weight_L, -1) + softplus(diag(weight_L)) * I      # [O,I,D,D], D=9
  w    = weight_loc + einsum('oiab,oib->oia', L, eps_w)         # [O,I,3,3]
  b    = bias_loc + eps_b * softplus(bias_ro)                   # [O]
  out  = conv2d(x, w, SAME, NCHW) + b
  with O = I = 64 channels, x [32, 64, 224, 224].

Distribution: data-parallel over the batch dim of x (32 images -> 8 cores x 4),
with the weight sampling replicated on every core (it is tiny).

Per-core kernel (row-parity conv, 75% PE utilization):
  - x is host-packed bf16 into SBUF layout [128, 113, 228]: partitions
    (parity q, in-channel), where q=0 slot k holds image row 2k and q=1 slot k
    holds row 2k-1 (staggered), columns padded by 2 on both sides.  Halo
    rows/columns are pre-zeroed on the host, so the kernel needs no memsets.
    Images stream in as 29-slot quarters through a 6-deep tile pool, so the
    HBM reads are consumption-paced instead of bursting against the output
    stores (HBM is ~358 GB/s per core).
  - output psum tiles are [128 = (row-parity p, out-channel), 2 pairs x 224]:
    out row 2k+p.  Per tile, 6 matmuls (2 input row-groups x 3 column shifts)
    apply all 9 taps exactly once per output: lhsT tiles have 3 of 4
    64x64 quadrants live (vs 2 of 4 for the image-paired block-diagonal
    scheme) -> 1.5x less PE time.
  - sampling: the host pre-layouts weight_L as a masked strict-lower
    [O, (b, i, a)] block plus the diagonal [O, (i, a)], and pre-broadcasts
    eps_w to the same shape (pure data reshuffle), so L @ eps is one
    contiguous VectorE multiply + 3 tree adds, and softplus(diag) is a
    contiguous ScalarE op.  The 9 tap matrices are PE-transposed into both
    partition halves at once (weights duplicated side by side), then 8
    batched strided copies assemble the 6 lhsT tiles.
  - ScalarE evacuates PSUM with the bias add fused (bf16 out); output is
    stored in a packed [8-strip, 128, 14, 224] layout the host re-interleaves.
"""

import sys
from contextlib import ExitStack

for _p in ("/opt/trn_rl_repo",):
    if _p not in sys.path:
        sys.path.insert(0, _p)

import numpy as np

import concourse.bass as bass
import concourse.bacc as bacc
import concourse.mybir as mybir
from concourse.tile import TileContext

F32 = mybir.dt.float32
F32R = mybir.dt.float32r
BF16 = mybir.dt.bfloat16
AF = mybir.ActivationFunctionType

N_CORES = 8
O = 64
I = 64
KK = 3
D = KK * KK  # 9
ID = I * D  # 576
HH = 224
WW = 224
NP = HH // 2 + 1  # 113 pair slots
WPAD = WW + 4  # 228: 2 zero cols each side
GSTRIP = 8  # output strips per image
PPG = (HH // 2) // GSTRIP  # 14 output row-pairs per strip
QS = [0, 28, 56, 84]  # x quarter start slots (29 slots each, 1-slot overlap)
QLEN = 29


def build_nc(nb=4, n_w1=26, n_w1b=6, n_w2=11):
    """Build the per-core Bass program. nb: images per core."""
    nc = bacc.Bacc("TRN2", target_bir_lowering=False, debug=False)

    xp_t = nc.dram_tensor("xp", [nb, 128, NP, WPAD], BF16, kind="ExternalInput").ap()
    wl2_t = nc.dram_tensor("wl2", [O, 8 * ID], BF16, kind="ExternalInput").ap()
    eps3_t = nc.dram_tensor("eps3", [O, 8 * ID], BF16, kind="ExternalInput").ap()
    diag_t = nc.dram_tensor("diag", [O, ID], BF16, kind="ExternalInput").ap()
    epsw_t = nc.dram_tensor("epsw", [O, ID], BF16, kind="ExternalInput").ap()
    wloc_t = nc.dram_tensor("wloc", [O, ID], BF16, kind="ExternalInput").ap()
    ident_t = nc.dram_tensor("ident", [O, O], F32, kind="ExternalInput").ap()
    identb_t = nc.dram_tensor("identb", [O, O], BF16, kind="ExternalInput").ap()
    bias3_t = nc.dram_tensor("bias3", [3, O], F32, kind="ExternalInput").ap()
    out_t = nc.dram_tensor(
        "out", [nb, GSTRIP, 128, PPG, WW], BF16, kind="ExternalOutput"
    ).ap()

    with TileContext(nc) as tc, ExitStack() as stack:
        # ---------------- weight + bias sampling (one-time prologue) --------
        cp = stack.enter_context(tc.tile_pool(name="consts", bufs=1))
        wl2 = cp.tile([O, 8 * ID], BF16, name="wl2", tag="wl2")
        eps3 = cp.tile([O, 8 * ID], BF16, name="eps3", tag="eps3")
        diag = cp.tile([O, ID], BF16, name="diag", tag="diag")
        epsw = cp.tile([O, ID], BF16, name="epsw_s", tag="epsw_s")
        wloc = cp.tile([O, ID], BF16, name="wloc_s", tag="wloc_s")
        ident = cp.tile([O, O], F32, name="ident_s", tag="ident_s")
        identb = cp.tile([O, O], BF16, name="identb_s", tag="identb_s")
        b3 = cp.tile([O, 3], F32, name="b3", tag="b3")
        b3p = cp.tile([3, O], F32, name="b3p", tag="b3p")
        sp = cp.tile([O, ID], BF16, name="sp", tag="sp")
        tmp = cp.tile([O, ID], BF16, name="tmp", tag="tmp")
        prod = cp.tile([O, 8 * ID], BF16, name="prod", tag="prod")
        wsamp = cp.tile([O, ID], BF16, name="wsamp", tag="wsamp")
        # sampled weights duplicated side by side: the tap transposes read
        # free dim (q, i) -> both partition halves of the [128, .] transpose
        # destination in one PE pass (no partition-shift DMA afterwards)
        wsampd = cp.tile([O, 2 * ID], BF16, name="wsampd", tag="wsampd")
        bias = cp.tile([128, 1], F32, name="bias", tag="bias")
        sp_b = cp.tile([O, 1], F32, name="sp_b", tag="sp_b")
        # 6 lhsT tiles side by side: [128, 6*128] = A_s (s=0..2), B_s (3..5)
        wts = cp.tile([128, 6 * 128], BF16, name="wts", tag="wts")

        # the three critical sampling tensors lead the sync ring in
        # consumption order (diag gates the softplus, wl2+eps3 the big
        # multiply); everything else trails or rides the scalar ring
        # sampling inputs split across both HWDGE rings so the two big
        # blocks (wl2, eps3) transfer in parallel ahead of everything else
        nc.sync.dma_start(wl2[:], wl2_t[:])
        nc.sync.dma_start(epsw[:], epsw_t[:])
        nc.sync.dma_start(ident[:], ident_t[:])
        nc.sync.dma_start(b3p[:], bias3_t[:])
        nc.scalar.dma_start(diag[:], diag_t[:])
        nc.scalar.dma_start(eps3[:], eps3_t[:])
        nc.scalar.dma_start(identb[:], identb_t[:])
        nc.scalar.dma_start(wloc[:], wloc_t[:])

        # PE warm-up feed: zero tiles via VectorE (fast, no SWDGE latency).
        # Full 128-partition matmuls: 64-wide ones do NOT trip the HAM clock
        # gate (measured: 90x [64,256] warmup left the PE at 1.2 GHz).
        identr = cp.tile([128, 128], F32R, name="identr", tag="identr")
        junk = cp.tile([128, 448], F32R, name="junk", tag="junk")
        with tc.high_priority():
            nc.vector.memset(identr[:].bitcast(F32), 0.0)
            nc.vector.memset(junk[:].bitcast(F32), 0.0)
        # zero the dead lhsT quadrants (A: q1/p1, B: q0/p0) in one shot
        nc.gpsimd.memset(wts[:].bitcast(F32), 0.0)



        with tc.tile_pool(name="prol", bufs=1, space="PSUM") as wp:
            # HAM needs ~3.4us of sustained full-width matmul activity to
            # lift the PE 1.2 -> 2.4 GHz; these also bridge PE-idle windows
            # while VectorE/ScalarE run the sampling chain.
            warm = wp.tile([128, 448], F32, name="warm")
            for k in range(n_w1):
                nc.tensor.matmul(
                    warm[:], identr[:], junk[:], start=(k == 0), stop=(k == n_w1 - 1)
                )

            # bias3 arrives as [3, 64]; transpose to [64, 3] on the PE (a
            # partition-major DMA of 64x3 elements costs ~17us in descriptors)
            bp_ps = wp.tile([O, 3], F32, name="bp_ps")
            nc.tensor.matmul(bp_ps[:], b3p[:], ident[0:3, 0:3], start=True, stop=True)

            for k in range(n_w1b):
                nc.tensor.matmul(
                    warm[:], identr[:], junk[:], start=(k == 0), stop=(k == n_w1b - 1)
                )

            # ---- VectorE sampling chain (all contiguous bf16) ------------
            # prod[o,(b,i,a)] = wl2 * eps3 elementwise; wl2 is host-masked
            # to the strict-lower taps and eps3 is eps_w host-broadcast over
            # a, so the b-tree-sum IS (tril(L,-1) @ eps).
            nc.vector.tensor_mul(prod[:], wl2[:], eps3[:])
            nc.vector.tensor_add(prod[:, 0 : 4 * ID], prod[:, 0 : 4 * ID],
                                 prod[:, 4 * ID : 8 * ID])
            nc.vector.tensor_add(prod[:, 0 : 2 * ID], prod[:, 0 : 2 * ID],
                                 prod[:, 2 * ID : 4 * ID])
            nc.vector.tensor_add(prod[:, 0:ID], prod[:, 0:ID], prod[:, ID : 2 * ID])
            nc.vector.tensor_copy(b3[:], bp_ps[:])

            # softplus(diag) on ScalarE: Exp then Ln (ln(e^x + 1)); there is
            # no Softplus LUT in this toolchain.
            nc.scalar.activation(sp[:], diag[:], AF.Exp)
            nc.scalar.activation(sp[:], sp[:], AF.Ln, bias=1.0)

            # wsamp = wloc + softplus(diag)*eps + strict_lower (twice, for
            # the both-halves transpose trick)
            nc.vector.tensor_mul(tmp[:], sp[:], epsw[:])
            nc.vector.tensor_add(wsamp[:], wloc[:], tmp[:])
            # both duplicate halves in one op: dst strides over the copies,
            # sources broadcast (stride 0)
            pstr = wsampd[:].ap[0][0]
            wd2 = bass.AP(
                tensor=wsampd[:].tensor, offset=wsampd[:].offset,
                ap=[[pstr, O], [ID, 2], [1, ID]],
            )
            ws_b = bass.AP(
                tensor=wsamp[:].tensor, offset=wsamp[:].offset,
                ap=[[wsamp[:].ap[0][0], O], [0, 2], [1, ID]],
            )
            pr_b = bass.AP(
                tensor=prod[:].tensor, offset=prod[:].offset,
                ap=[[prod[:].ap[0][0], O], [0, 2], [1, ID]],
            )
            nc.vector.tensor_tensor(wd2, ws_b, pr_b, mybir.AluOpType.add)

            # ---- tap transposes + lhsT assembly --------------------------
            # T[t][ich,och] = wsamp[och, ich*9+t], written to BOTH partition
            # halves of ptA/ptB at once via the duplicated wsampd free dim.
            ptA = wp.tile([128, 5 * O], BF16, name="ptA")
            ptB = wp.tile([128, 4 * O], BF16, name="ptB")
            for a in range(D):
                w_a2 = bass.AP(
                    tensor=wsampd[:].tensor,
                    offset=wsampd[:].offset + a,
                    ap=[list(p) for p in wsampd[:].ap[:1]] + [[ID, 2], [D, I]],
                )
                dst_pt = ptA if a < 5 else ptB
                c = a if a < 5 else a - 5
                nc.tensor.matmul(
                    dst_pt[:, c * O : (c + 1) * O],
                    w_a2,
                    identb[:],
                    is_transpose=True,
                    start=(c == 0),
                    stop=(c == (4 if a < 5 else 3)),
                    skip_group_check=True,
                )

            # keep the PE busy while the lhsT copies run
            for k in range(n_w2):
                nc.tensor.matmul(
                    warm[:], identr[:], junk[:], start=(k == 0), stop=(k == n_w2 - 1)
                )

            # batched strided copies (dst stride 128, src stride 64):
            #   A_s: [q0,p0]=T[3+s]  [q0,p1]=T[s]  [q1,p0]=T[s]  [q1,p1]=0
            #   B_s: [q0,p0]=0  [q0,p1]=T[6+s]  [q1,p0]=T[6+s]  [q1,p1]=T[3+s]
            def bcopy(eng, dst_c0, dst_n, src_pt, src_half, src_c0):
                pstr = wts[:].ap[0][0]
                dst = bass.AP(
                    tensor=wts[:].tensor,
                    offset=wts[:].offset + src_half * 64 * pstr + dst_c0,
                    ap=[[pstr, 64], [128, dst_n], [1, O]],
                )
                s_ = src_pt[src_half * 64 : src_half * 64 + 64,
                            src_c0 : src_c0 + dst_n * O]
                src = bass.AP(
                    tensor=s_.tensor, offset=s_.offset,
                    ap=[list(s_.ap[0])] + [[O, dst_n], [1, O]],
                )
                if eng == "v":
                    nc.vector.tensor_copy(dst, src)
                else:
                    nc.scalar.activation(dst, src, AF.Copy)

            bcopy("v", O, 3, ptA, 0, 0)        # A q0,p1 <- T[0..2]
            bcopy("s", 0, 3, ptA, 1, 0)        # A q1,p0 <- T[0..2]
            bcopy("v", 0, 2, ptA, 0, 3 * O)    # A q0,p0 <- T[3..4]
            bcopy("s", 2 * 128, 1, ptB, 0, 0)  # A2 q0,p0 <- T[5]
            bcopy("v", 3 * 128 + O, 3, ptB, 0, O)      # B q0,p1 <- T[6..8]
            bcopy("s", 3 * 128, 3, ptB, 1, O)          # B q1,p0 <- T[6..8]
            bcopy("v", 3 * 128 + O, 2, ptA, 1, 3 * O)  # B0-1 q1,p1 <- T[3..4]
            bcopy("s", 5 * 128 + O, 1, ptB, 1, 0)      # B2 q1,p1 <- T[5]

            # bias = bias_loc + eps_b * softplus(bias_ro)  (off critical path)
            nc.scalar.activation(sp_b[:], b3[:, 1:2], AF.Exp)
            nc.scalar.activation(sp_b[:], sp_b[:], AF.Ln, bias=1.0)
            nc.vector.tensor_mul(sp_b[:], sp_b[:], b3[:, 2:3])
            nc.vector.tensor_add(bias[0:O, :], b3[:, 0:1], sp_b[:])
            nc.scalar.dma_start(bias[O:128, :], bias[0:O, :])

        # ---------------- convolution ---------------------------------------
        # per psum tile t (out rows 4t..4t+3 of one image):
        #   acc[(p,och), (k in {2t,2t+1}, c)] = out row 2k+p
        #   A_s: rhs slots (2t, 2t+1)   B_s: rhs slots (2t+1, 2t+2)
        #   rhs col start = s+1 (packed col cc = image col + 2)
        # bufs=2: only two quarters in flight, so image reads are paced by
        # conv consumption instead of queueing 6.4MB against the stores and
        # the prologue loads (whole-packet round-robin on the SDMA engines)
        xqp = stack.enter_context(tc.tile_pool(name="xq", bufs=2))
        op = stack.enter_context(tc.tile_pool(name="ostrip", bufs=2))
        pp = stack.enter_context(tc.tile_pool(name="acc", bufs=8, space="PSUM"))
        for n in range(nb):
            xq = []
            for q in range(4):
                xt = xqp.tile([128, QLEN, WPAD], BF16, name="xq")
                nc.sync.dma_start(xt[:], xp_t[n, :, QS[q] : QS[q] + QLEN, :])
                xq.append(xt)
            for g in range(GSTRIP):
                os_ = op.tile([128, PPG, WW], BF16, name="os_")
                last = n == nb - 1 and g == GSTRIP - 1
                # psum tiles in pairs, matmuls grouped by lhsT so each
                # stationary load serves two consecutive matmuls
                for prt in ((0, 1), (2, 3), (4, 5), (6,)):
                    accs = []
                    for tt in prt:
                        t = (PPG // 2) * g + tt
                        qi = (t >= 14) + (t >= 28) + (t >= 42)
                        accs.append(
                            (tt, t, xq[qi], 2 * t - QS[qi],
                             pp.tile([128, 2, WW], F32, name="acc"))
                        )
                    for s in range(3):
                        for lhs_c0, off, is_a in (
                            (s * 128, 0, True),
                            ((3 + s) * 128, 1, False),
                        ):
                            for tt, t, xs, lo, acc in accs:
                                nc.tensor.matmul(
                                    acc[:],
                                    wts[:, lhs_c0 : lhs_c0 + 128],
                                    xs[:, lo + off : lo + off + 2,
                                       s + 1 : s + 1 + WW],
                                    start=(s == 0 and is_a),
                                    stop=(s == 2 and not is_a),
                                    skip_group_check=True,
                                )
                    for tt, t, xs, lo, acc in accs:
                        nc.scalar.activation(
                            os_[:, 2 * tt : 2 * tt + 2, :],
                            acc[:],
                            AF.Identity,
                            bias=bias[:, 0:1],
                        )
                        # taper: stream the final strip out in pieces so the
                        # kernel does not end on a full-size store
                        if last and tt in (1, 3, 5):
                            k0, k1 = {1: (0, 4), 3: (4, 8), 5: (8, 12)}[tt]
                            nc.scalar.dma_start(
                                out_t[n, g, :, k0:k1, :], os_[:, k0:k1, :]
                            )
                if last:
                    nc.scalar.dma_start(out_t[n, g, :, 12:PPG, :], os_[:, 12:PPG, :])
                else:
                    nc.scalar.dma_start(out_t[n, g], os_[:])

    nc.compile()
    return nc


_CACHED_NC = None


def _pack_x(x_shard_bf):
    """[nb, 64, 224, 224] bf16 -> [nb, 128, 113, 228] staggered parity pack."""
    nb = x_shard_bf.shape[0]
    xp = np.zeros((nb, 128, NP, WPAD), dtype=x_shard_bf.dtype)
    xp[:, 0:64, 0 : HH // 2, 2 : WW + 2] = x_shard_bf[:, :, 0::2, :]
    xp[:, 64:128, 1 : HH // 2 + 1, 2 : WW + 2] = x_shard_bf[:, :, 1::2, :]
    return xp


def _host_inputs(x_shard, weight_loc, weight_L, bias_loc, bias_ro, eps_w, eps_b):
    import ml_dtypes

    bf = ml_dtypes.bfloat16
    wlf = np.asarray(weight_L, np.float32)  # [O, I, D(a), D(b)]
    mask = np.tril(np.ones((D, D), np.float32), -1)  # [a, b]: a > b
    wl2 = (wlf * mask).transpose(0, 3, 1, 2)[:, 0:8]  # [O, b, I, a]
    dg = np.diagonal(wlf, axis1=2, axis2=3)  # [O, I, D]
    ew = np.asarray(eps_w, np.float32)  # [O, I, D(b)]
    eps3 = np.broadcast_to(
        ew.transpose(0, 2, 1)[:, 0:8, :, None], (O, 8, I, D)
    )  # [O, b, I, a]: eps_w[o,i,b] for every a
    return {
        "xp": _pack_x(np.asarray(x_shard).astype(bf)),
        "wl2": np.ascontiguousarray(wl2.reshape(O, 8 * ID)).astype(bf),
        "eps3": np.ascontiguousarray(eps3.reshape(O, 8 * ID)).astype(bf),
        "diag": np.ascontiguousarray(dg.reshape(O, ID)).astype(bf),
        "epsw": np.ascontiguousarray(ew.reshape(O, ID)).astype(bf),
        "wloc": np.ascontiguousarray(
            np.asarray(weight_loc, np.float32).reshape(O, ID)
        ).astype(bf),
        "ident": np.eye(O, dtype=np.float32),
        "identb": np.eye(O, dtype=np.float32).astype(bf),
        "bias3": np.ascontiguousarray(
            np.stack([bias_loc, bias_ro, eps_b]).astype(np.float32)
        ),
    }


def kernel(x, weight_loc, weight_L, bias_loc, bias_ro, eps_w, eps_b):
    global _CACHED_NC
    from concourse.bass_utils import run_bass_kernel_spmd

    x = np.asarray(x, np.float32)
    nb = x.shape[0] // N_CORES
    if _CACHED_NC is None:
        _CACHED_NC = build_nc(nb=nb)
    nc = _CACHED_NC

    import ml_dtypes

    x_bf = x.astype(ml_dtypes.bfloat16)
    in_maps = [
        _host_inputs(
            x_bf[c * nb : (c + 1) * nb],
            np.asarray(weight_loc),
            np.asarray(weight_L),
            np.asarray(bias_loc),
            np.asarray(bias_ro),
            np.asarray(eps_w),
            np.asarray(eps_b),
        )
        for c in range(N_CORES)
    ]
    res = run_bass_kernel_spmd(nc, in_maps, list(range(N_CORES)))
    outs = []
    for c in range(N_CORES):
        o = np.asarray(res.results[c]["out"])  # [nb, 8, 128, 14, 224] bf16
        o = o.reshape(nb, GSTRIP, 2, O, PPG, WW).transpose(0, 3, 1, 4, 2, 5)
        outs.append(o.reshape(nb, O, HH, WW).astype(np.float32))
    return np.concatenate(outs, axis=0)


# revision 47
# speedup vs baseline: 1.1848x; 1.0010x over previous
"""Trainium2 Bass kernel for nn_Conv2DExperimental (MVN-sampled 3x3 conv).

Computation (per the nn.Module):
  L    = tril(weight_L, -1) + softplus(diag(weight_L)) * I      # [O,I,D,D], D=9
  w    = weight_loc + einsum('oiab,oib->oia', L, eps_w)         # [O,I,3,3]
  b    = bias_loc + eps_b * softplus(bias_ro)                   # [O]
  out  = conv2d(x, w, SAME, NCHW) + b
  with O = I = 64 channels, x [32, 64, 224, 224].

Distribution: data-parallel over the batch dim of x (32 images -> 8 cores x 4),
with the weight sampling replicated on every core (it is tiny).

Per-core kernel (row-parity conv, 75% PE utilization):
  - x is host-packed bf16 into SBUF layout [128, 113, 228]: partitions
    (parity q, in-channel), where q=0 slot k holds image row 2k and q=1 slot k
    holds row 2k-1 (staggered), columns padded by 2 on both sides.  Halo
    rows/columns are pre-zeroed on the host, so the kernel needs no memsets.
    Images stream in as 29-slot quarters through a 6-deep tile pool, so the
    HBM reads are consumption-paced instead of bursting against the output
    stores (HBM is ~358 GB/s per core).
  - output psum tiles are [128 = (row-parity p, out-channel), 2 pairs x 224]:
    out row 2k+p.  Per tile, 6 matmuls (2 input row-groups x 3 column shifts)
    apply all 9 taps exactly once per output: lhsT tiles have 3 of 4
    64x64 quadrants live (vs 2 of 4 for the image-paired block-diagonal
    scheme) -> 1.5x less PE time.
  - sampling: the host pre-layouts weight_L as a masked strict-lower
    [O, (b, i, a)] block plus the diagonal [O, (i, a)], and pre-broadcasts
    eps_w to the same shape (pure data reshuffle), so L @ eps is one
    contiguous VectorE multiply + 3 tree adds, and softplus(diag) is a
    contiguous ScalarE op.  The 9 tap matrices are PE-transposed into both
    partition halves at once (weights duplicated side by side), then 8
    batched strided copies assemble the 6 lhsT tiles.
  - ScalarE evacuates PSUM with the bias add fused (bf16 out); output is
    stored in a packed [8-strip, 128, 14, 224] layout the host re-interleaves.
"""

import sys
from contextlib import ExitStack

for _p in ("/opt/trn_rl_repo",):
    if _p not in sys.path:
        sys.path.insert(0, _p)

import numpy as np

import concourse.bass as bass
import concourse.bacc as bacc
import concourse.mybir as mybir
from concourse.tile import TileContext

F32 = mybir.dt.float32
F32R = mybir.dt.float32r
BF16 = mybir.dt.bfloat16
AF = mybir.ActivationFunctionType

N_CORES = 8
O = 64
I = 64
KK = 3
D = KK * KK  # 9
ID = I * D  # 576
HH = 224
WW = 224
NP = HH // 2 + 1  # 113 pair slots
WPAD = WW + 4  # 228: 2 zero cols each side
GSTRIP = 8  # output strips per image
PPG = (HH // 2) // GSTRIP  # 14 output row-pairs per strip
QS = [0, 28, 56, 84]  # x quarter start slots (29 slots each, 1-slot overlap)
QLEN = 29


def build_nc(nb=4, n_w1=26, n_w1b=6, n_w2=11):
    """Build the per-core Bass program. nb: images per core."""
    nc = bacc.Bacc("TRN2", target_bir_lowering=False, debug=False)

    xp_t = nc.dram_tensor("xp", [nb, 128, NP, WPAD], BF16, kind="ExternalInput").ap()
    wl2_t = nc.dram_tensor("wl2", [O, 8 * ID], BF16, kind="ExternalInput").ap()
    eps3_t = nc.dram_tensor("eps3", [O, 8 * ID], BF16, kind="ExternalInput").ap()
    diag_t = nc.dram_tensor("diag", [O, ID], BF16, kind="ExternalInput").ap()
    epsw_t = nc.dram_tensor("epsw", [O, ID], BF16, kind="ExternalInput").ap()
    wloc_t = nc.dram_tensor("wloc", [O, ID], BF16, kind="ExternalInput").ap()
    ident_t = nc.dram_tensor("ident", [O, O], F32, kind="ExternalInput").ap()
    identb_t = nc.dram_tensor("identb", [O, O], BF16, kind="ExternalInput").ap()
    bias3_t = nc.dram_tensor("bias3", [3, O], F32, kind="ExternalInput").ap()
    out_t = nc.dram_tensor(
        "out", [nb, GSTRIP, 128, PPG, WW], BF16, kind="ExternalOutput"
    ).ap()

    with TileContext(nc) as tc, ExitStack() as stack:
        # ---------------- weight + bias sampling (one-time prologue) --------
        cp = stack.enter_context(tc.tile_pool(name="consts", bufs=1))
        wl2 = cp.tile([O, 8 * ID], BF16, name="wl2", tag="wl2")
        eps3 = cp.tile([O, 8 * ID], BF16, name="eps3", tag="eps3")
        diag = cp.tile([O, ID], BF16, name="diag", tag="diag")
        epsw = cp.tile([O, ID], BF16, name="epsw_s", tag="epsw_s")
        wloc = cp.tile([O, ID], BF16, name="wloc_s", tag="wloc_s")
        ident = cp.tile([O, O], F32, name="ident_s", tag="ident_s")
        identb = cp.tile([O, O], BF16, name="identb_s", tag="identb_s")
        b3 = cp.tile([O, 3], F32, name="b3", tag="b3")
        b3p = cp.tile([3, O], F32, name="b3p", tag="b3p")
        sp = cp.tile([O, ID], BF16, name="sp", tag="sp")
        tmp = cp.tile([O, ID], BF16, name="tmp", tag="tmp")
        prod = cp.tile([O, 8 * ID], BF16, name="prod", tag="prod")
        wsamp = cp.tile([O, ID], BF16, name="wsamp", tag="wsamp")
        # sampled weights duplicated side by side: the tap transposes read
        # free dim (q, i) -> both partition halves of the [128, .] transpose
        # destination in one PE pass (no partition-shift DMA afterwards)
        wsampd = cp.tile([O, 2 * ID], BF16, name="wsampd", tag="wsampd")
        bias = cp.tile([128, 1], F32, name="bias", tag="bias")
        sp_b = cp.tile([O, 1], F32, name="sp_b", tag="sp_b")
        # 6 lhsT tiles side by side: [128, 6*128] = A_s (s=0..2), B_s (3..5)
        wts = cp.tile([128, 6 * 128], BF16, name="wts", tag="wts")

        # the three critical sampling tensors lead the sync ring in
        # consumption order (diag gates the softplus, wl2+eps3 the big
        # multiply); everything else trails or rides the scalar ring
        # sampling inputs split across both HWDGE rings so the two big
        # blocks (wl2, eps3) transfer in parallel ahead of everything else
        nc.sync.dma_start(wl2[:], wl2_t[:])
        nc.sync.dma_start(epsw[:], epsw_t[:])
        nc.sync.dma_start(ident[:], ident_t[:])
        nc.sync.dma_start(b3p[:], bias3_t[:])
        nc.scalar.dma_start(diag[:], diag_t[:])
        nc.scalar.dma_start(eps3[:], eps3_t[:])
        nc.scalar.dma_start(identb[:], identb_t[:])
        nc.scalar.dma_start(wloc[:], wloc_t[:])

        # PE warm-up feed: zero tiles via VectorE (fast, no SWDGE latency).
        # Full 128-partition matmuls: 64-wide ones do NOT trip the HAM clock
        # gate (measured: 90x [64,256] warmup left the PE at 1.2 GHz).
        identr = cp.tile([128, 128], F32R, name="identr", tag="identr")
        junk = cp.tile([128, 448], F32R, name="junk", tag="junk")
        with tc.high_priority():
            nc.vector.memset(identr[:].bitcast(F32), 0.0)
            nc.vector.memset(junk[:].bitcast(F32), 0.0)
        # zero the dead lhsT quadrants (A: q1/p1, B: q0/p0) in one shot
        nc.gpsimd.memset(wts[:].bitcast(F32), 0.0)



        with tc.tile_pool(name="prol", bufs=1, space="PSUM") as wp:
            # HAM needs ~3.4us of sustained full-width matmul activity to
            # lift the PE 1.2 -> 2.4 GHz; these also bridge PE-idle windows
            # while VectorE/ScalarE run the sampling chain.
            warm = wp.tile([128, 448], F32, name="warm")
            for k in range(n_w1):
                nc.tensor.matmul(
                    warm[:], identr[:], junk[:], start=(k == 0), stop=(k == n_w1 - 1)
                )

            # bias3 arrives as [3, 64]; transpose to [64, 3] on the PE (a
            # partition-major DMA of 64x3 elements costs ~17us in descriptors)
            bp_ps = wp.tile([O, 3], F32, name="bp_ps")
            nc.tensor.matmul(bp_ps[:], b3p[:], ident[0:3, 0:3], start=True, stop=True)

            for k in range(n_w1b):
                nc.tensor.matmul(
                    warm[:], identr[:], junk[:], start=(k == 0), stop=(k == n_w1b - 1)
                )

            # ---- VectorE sampling chain (all contiguous bf16) ------------
            # prod[o,(b,i,a)] = wl2 * eps3 elementwise; wl2 is host-masked
            # to the strict-lower taps and eps3 is eps_w host-broadcast over
            # a, so the b-tree-sum IS (tril(L,-1) @ eps).
            nc.vector.tensor_mul(prod[:], wl2[:], eps3[:])
            nc.vector.tensor_add(prod[:, 0 : 4 * ID], prod[:, 0 : 4 * ID],
                                 prod[:, 4 * ID : 8 * ID])
            nc.vector.tensor_add(prod[:, 0 : 2 * ID], prod[:, 0 : 2 * ID],
                                 prod[:, 2 * ID : 4 * ID])
            nc.vector.tensor_add(prod[:, 0:ID], prod[:, 0:ID], prod[:, ID : 2 * ID])
            nc.vector.tensor_copy(b3[:], bp_ps[:])

            # softplus(diag) on ScalarE: Exp then Ln (ln(e^x + 1)); there is
            # no Softplus LUT in this toolchain.
            nc.scalar.activation(sp[:], diag[:], AF.Exp)
            nc.scalar.activation(sp[:], sp[:], AF.Ln, bias=1.0)

            # wsamp = wloc + softplus(diag)*eps + strict_lower (twice, for
            # the both-halves transpose trick)
            nc.vector.tensor_mul(tmp[:], sp[:], epsw[:])
            nc.vector.tensor_add(wsamp[:], wloc[:], tmp[:])
            # both duplicate halves in one op: dst strides over the copies,
            # sources broadcast (stride 0)
            pstr = wsampd[:].ap[0][0]
            wd2 = bass.AP(
                tensor=wsampd[:].tensor, offset=wsampd[:].offset,
                ap=[[pstr, O], [ID, 2], [1, ID]],
            )
            ws_b = bass.AP(
                tensor=wsamp[:].tensor, offset=wsamp[:].offset,
                ap=[[wsamp[:].ap[0][0], O], [0, 2], [1, ID]],
            )
            pr_b = bass.AP(
                tensor=prod[:].tensor, offset=prod[:].offset,
                ap=[[prod[:].ap[0][0], O], [0, 2], [1, ID]],
            )
            nc.vector.tensor_tensor(wd2, ws_b, pr_b, mybir.AluOpType.add)

            # ---- tap transposes + lhsT assembly --------------------------
            # T[t][ich,och] = wsamp[och, ich*9+t], written to BOTH partition
            # halves of ptA/ptB at once via the duplicated wsampd free dim.
            ptA = wp.tile([128, 5 * O], BF16, name="ptA")
            ptB = wp.tile([128, 4 * O], BF16, name="ptB")
            for a in range(D):
                w_a2 = bass.AP(
                    tensor=wsampd[:].tensor,
                    offset=wsampd[:].offset + a,
                    ap=[list(p) for p in wsampd[:].ap[:1]] + [[ID, 2], [D, I]],
                )
                dst_pt = ptA if a < 5 else ptB
                c = a if a < 5 else a - 5
                nc.tensor.matmul(
                    dst_pt[:, c * O : (c + 1) * O],
                    w_a2,
                    identb[:],
                    is_transpose=True,
                    start=(c == 0),
                    stop=(c == (4 if a < 5 else 3)),
                    skip_group_check=True,
                )

            # keep the PE busy while the lhsT copies run
            for k in range(n_w2):
                nc.tensor.matmul(
                    warm[:], identr[:], junk[:], start=(k == 0), stop=(k == n_w2 - 1)
                )

            # batched strided copies (dst stride 128, src stride 64):
            #   A_s: [q0,p0]=T[3+s]  [q0,p1]=T[s]  [q1,p0]=T[s]  [q1,p1]=0
            #   B_s: [q0,p0]=0  [q0,p1]=T[6+s]  [q1,p0]=T[6+s]  [q1,p1]=T[3+s]
            def bcopy(eng, dst_c0, dst_n, src_pt, src_half, src_c0):
                pstr = wts[:].ap[0][0]
                dst = bass.AP(
                    tensor=wts[:].tensor,
                    offset=wts[:].offset + src_half * 64 * pstr + dst_c0,
                    ap=[[pstr, 64], [128, dst_n], [1, O]],
                )
                s_ = src_pt[src_half * 64 : src_half * 64 + 64,
                            src_c0 : src_c0 + dst_n * O]
                src = bass.AP(
                    tensor=s_.tensor, offset=s_.offset,
                    ap=[list(s_.ap[0])] + [[O, dst_n], [1, O]],
                )
                if eng == "v":
                    nc.vector.tensor_copy(dst, src)
                else:
                    nc.scalar.activation(dst, src, AF.Copy)

            bcopy("v", O, 3, ptA, 0, 0)        # A q0,p1 <- T[0..2]
            bcopy("v", 0, 3, ptA, 1, 0)        # A q1,p0 <- T[0..2]
            bcopy("v", 0, 2, ptA, 0, 3 * O)    # A q0,p0 <- T[3..4]
            bcopy("v", 2 * 128, 1, ptB, 0, 0)  # A2 q0,p0 <- T[5]
            bcopy("v", 3 * 128 + O, 3, ptB, 0, O)      # B q0,p1 <- T[6..8]
            bcopy("v", 3 * 128, 3, ptB, 1, O)          # B q1,p0 <- T[6..8]
            bcopy("v", 3 * 128 + O, 2, ptA, 1, 3 * O)  # B0-1 q1,p1 <- T[3..4]
            bcopy("v", 5 * 128 + O, 1, ptB, 1, 0)      # B2 q1,p1 <- T[5]

            # bias = bias_loc + eps_b * softplus(bias_ro)  (off critical path)
            nc.scalar.activation(sp_b[:], b3[:, 1:2], AF.Exp)
            nc.scalar.activation(sp_b[:], sp_b[:], AF.Ln, bias=1.0)
            nc.vector.tensor_mul(sp_b[:], sp_b[:], b3[:, 2:3])
            nc.vector.tensor_add(bias[0:O, :], b3[:, 0:1], sp_b[:])
            nc.scalar.dma_start(bias[O:128, :], bias[0:O, :])

        # ---------------- convolution ---------------------------------------
        # per psum tile t (out rows 4t..4t+3 of one image):
        #   acc[(p,och), (k in {2t,2t+1}, c)] = out row 2k+p
        #   A_s: rhs slots (2t, 2t+1)   B_s: rhs slots (2t+1, 2t+2)
        #   rhs col start = s+1 (packed col cc = image col + 2)
        # bufs=2: only two quarters in flight, so image reads are paced by
        # conv consumption instead of queueing 6.4MB against the stores and
        # the prologue loads (whole-packet round-robin on the SDMA engines)
        xqp = stack.enter_context(tc.tile_pool(name="xq", bufs=2))
        op = stack.enter_context(tc.tile_pool(name="ostrip", bufs=2))
        pp = stack.enter_context(tc.tile_pool(name="acc", bufs=8, space="PSUM"))
        for n in range(nb):
            xq = []
            for q in range(4):
                xt = xqp.tile([128, QLEN, WPAD], BF16, name="xq")
                nc.sync.dma_start(xt[:], xp_t[n, :, QS[q] : QS[q] + QLEN, :])
                xq.append(xt)
            for g in range(GSTRIP):
                os_ = op.tile([128, PPG, WW], BF16, name="os_")
                last = n == nb - 1 and g == GSTRIP - 1
                # psum tiles in pairs, matmuls grouped by lhsT so each
                # stationary load serves two consecutive matmuls
                for prt in ((0, 1), (2, 3), (4, 5), (6,)):
                    accs = []
                    for tt in prt:
                        t = (PPG // 2) * g + tt
                        qi = (t >= 14) + (t >= 28) + (t >= 42)
                        accs.append(
                            (tt, t, xq[qi], 2 * t - QS[qi],
                             pp.tile([128, 2, WW], F32, name="acc"))
                        )
                    for s in range(3):
                        for lhs_c0, off, is_a in (
                            (s * 128, 0, True),
                            ((3 + s) * 128, 1, False),
                        ):
                            for tt, t, xs, lo, acc in accs:
                                nc.tensor.matmul(
                                    acc[:],
                                    wts[:, lhs_c0 : lhs_c0 + 128],
                                    xs[:, lo + off : lo + off + 2,
                                       s + 1 : s + 1 + WW],
                                    start=(s == 0 and is_a),
                                    stop=(s == 2 and not is_a),
                                    skip_group_check=True,
                                )
                    for tt, t, xs, lo, acc in accs:
                        nc.scalar.activation(
                            os_[:, 2 * tt : 2 * tt + 2, :],
                            acc[:],
                            AF.Identity,
                            bias=bias[:, 0:1],
                        )
                        # taper: stream the final strip out in pieces so the
                        # kernel does not end on a full-size store
                        if last and tt in (1, 3, 5):
                            k0, k1 = {1: (0, 4), 3: (4, 8), 5: (8, 12)}[tt]
                            nc.scalar.dma_start(
                                out_t[n, g, :, k0:k1, :], os_[:, k0:k1, :]
                            )
                if last:
                    nc.scalar.dma_start(out_t[n, g, :, 12:PPG, :], os_[:, 12:PPG, :])
                else:
                    nc.scalar.dma_start(out_t[n, g], os_[:])

    nc.compile()
    return nc


_CACHED_NC = None


def _pack_x(x_shard_bf):
    """[nb, 64, 224, 224] bf16 -> [nb, 128, 113, 228] staggered parity pack."""
    nb = x_shard_bf.shape[0]
    xp = np.zeros((nb, 128, NP, WPAD), dtype=x_shard_bf.dtype)
    xp[:, 0:64, 0 : HH // 2, 2 : WW + 2] = x_shard_bf[:, :, 0::2, :]
    xp[:, 64:128, 1 : HH // 2 + 1, 2 : WW + 2] = x_shard_bf[:, :, 1::2, :]
    return xp


def _host_inputs(x_shard, weight_loc, weight_L, bias_loc, bias_ro, eps_w, eps_b):
    import ml_dtypes

    bf = ml_dtypes.bfloat16
    wlf = np.asarray(weight_L, np.float32)  # [O, I, D(a), D(b)]
    mask = np.tril(np.ones((D, D), np.float32), -1)  # [a, b]: a > b
    wl2 = (wlf * mask).transpose(0, 3, 1, 2)[:, 0:8]  # [O, b, I, a]
    dg = np.diagonal(wlf, axis1=2, axis2=3)  # [O, I, D]
    ew = np.asarray(eps_w, np.float32)  # [O, I, D(b)]
    eps3 = np.broadcast_to(
        ew.transpose(0, 2, 1)[:, 0:8, :, None], (O, 8, I, D)
    )  # [O, b, I, a]: eps_w[o,i,b] for every a
    return {
        "xp": _pack_x(np.asarray(x_shard).astype(bf)),
        "wl2": np.ascontiguousarray(wl2.reshape(O, 8 * ID)).astype(bf),
        "eps3": np.ascontiguousarray(eps3.reshape(O, 8 * ID)).astype(bf),
        "diag": np.ascontiguousarray(dg.reshape(O, ID)).astype(bf),
        "epsw": np.ascontiguousarray(ew.reshape(O, ID)).astype(bf),
        "wloc": np.ascontiguousarray(
            np.asarray(weight_loc, np.float32).reshape(O, ID)
        ).astype(bf),
        "ident": np.eye(O, dtype=np.float32),
        "identb": np.eye(O, dtype=np.float32).astype(bf),
        "bias3": np.ascontiguousarray(
            np.stack([bias_loc, bias_ro, eps_b]).astype(np.float32)
        ),
    }


def kernel(x, weight_loc, weight_L, bias_loc, bias_ro, eps_w, eps_b):
    global _CACHED_NC
    from concourse.bass_utils import run_bass_kernel_spmd

    x = np.asarray(x, np.float32)
    nb = x.shape[0] // N_CORES
    if _CACHED_NC is None:
        _CACHED_NC = build_nc(nb=nb)
    nc = _CACHED_NC

    import ml_dtypes

    x_bf = x.astype(ml_dtypes.bfloat16)
    in_maps = [
        _host_inputs(
            x_bf[c * nb : (c + 1) * nb],
            np.asarray(weight_loc),
            np.asarray(weight_L),
            np.asarray(bias_loc),
            np.asarray(bias_ro),
            np.asarray(eps_w),
            np.asarray(eps_b),
        )
        for c in range(N_CORES)
    ]
    res = run_bass_kernel_spmd(nc, in_maps, list(range(N_CORES)))
    outs = []
    for c in range(N_CORES):
        o = np.asarray(res.results[c]["out"])  # [nb, 8, 128, 14, 224] bf16
        o = o.reshape(nb, GSTRIP, 2, O, PPG, WW).transpose(0, 3, 1, 4, 2, 5)
        outs.append(o.reshape(nb, O, HH, WW).astype(np.float32))
    return np.concatenate(outs, axis=0)
